# revision 1
# baseline (speedup 1.0000x reference)
"""Trainium2 Bass kernel for nn_LSTMClassifier (B=256,T=1024,D=64,H=128,C=10).

Strategy: data-parallel over batch across 8 cores (32 seqs/core).
Per-core layout is gate-major: partitions = hidden units, batch on the
free dim, so h^T [128,32] is born in the right layout to be the moving
operand of the next step's gate matmuls (no per-step transposes).

Per layer the input transform xg = W_ih @ x + b is computed by chunked
GEMMs directly INTO the PSUM banks that the per-step recurrence matmuls
then accumulate onto (start=False), so no separate xg add is needed.
L0's bias rides a K=65 augmented stationary (ones row in x^T); L1's bias
is one rank-4 indicator matmul per bank. The two layers are interleaved
with a 2-bank lag so their dependency chains overlap on the engines.
"""

import os
import sys

import numpy as np

for _p in ("/opt/trn_rl_repo",):
    if _p not in sys.path:
        sys.path.insert(0, _p)

import ml_dtypes  # noqa: E402

B, T, D, H, C = 256, 1024, 64, 128, 10
NCORES, BL = 8, 32
# column-block order [i, f, o, g]; reference split order is (i, f, g, o)
PERM = [0, 1, 3, 2]
LAG = 2  # banks (of 4 steps) that L1 trails L0 in program order

_cache = {}


def _build_nc(t_steps):
    from contextlib import ExitStack

    import concourse.bass as bass
    import concourse.mybir as mybir
    from concourse import bacc
    from concourse.tile import TileContext

    dt = mybir.dt
    AF = mybir.ActivationFunctionType
    MS = bass.MemorySpace

    nc = bacc.Bacc(None, target_bir_lowering=False, debug=False)
    NB = t_steps // 4

    xta_d = nc.dram_tensor("xta", [D + 1, t_steps * BL], dt.bfloat16, kind="ExternalInput")
    w0aug_d = nc.dram_tensor("w0aug", [D + 1, 512], dt.bfloat16, kind="ExternalInput")
    whh0_d = nc.dram_tensor("whh0t", [H, 512], dt.bfloat16, kind="ExternalInput")
    w1_d = nc.dram_tensor("w1t", [H, 512], dt.bfloat16, kind="ExternalInput")
    whh1_d = nc.dram_tensor("whh1t", [H, 512], dt.bfloat16, kind="ExternalInput")
    b1_d = nc.dram_tensor("b1row", [4, H], dt.bfloat16, kind="ExternalInput")
    ind_d = nc.dram_tensor("ind", [4, 512], dt.bfloat16, kind="ExternalInput")
    whead_d = nc.dram_tensor("wheadt", [H, 16], dt.bfloat16, kind="ExternalInput")
    bhead_d = nc.dram_tensor("bhead", [16, 1], dt.float32, kind="ExternalInput")
    out_d = nc.dram_tensor("out", [16, BL], dt.float32, kind="ExternalOutput")
    dbg = os.environ.get("KDBG") == "1"
    if dbg:
        h1_d = nc.dram_tensor("h1dump", [H, t_steps * BL], dt.bfloat16, kind="ExternalOutput")
        bank_d = nc.dram_tensor("bankdump", [H, 512], dt.float32, kind="ExternalOutput")
        sig_d = nc.dram_tensor("sigdump", [H, 128], dt.float32, kind="ExternalOutput")

    with TileContext(nc) as tc, ExitStack() as ctx:
        consts = ctx.enter_context(tc.tile_pool(name="consts", bufs=1))
        xta = consts.tile([D + 1, t_steps * BL], dt.bfloat16, tag="xta")
        w0aug = consts.tile([D + 1, 512], dt.bfloat16, tag="w0aug")
        whh0 = consts.tile([H, 512], dt.bfloat16, tag="whh0")
        w1 = consts.tile([H, 512], dt.bfloat16, tag="w1")
        whh1 = consts.tile([H, 512], dt.bfloat16, tag="whh1")
        b1row = consts.tile([4, H], dt.bfloat16, tag="b1row")
        ind = consts.tile([4, 512], dt.bfloat16, tag="ind")
        wheadt = consts.tile([H, 16], dt.bfloat16, tag="wheadt")
        bhead = consts.tile([16, 1], dt.float32, tag="bhead")
        h1T = consts.tile([H, t_steps, BL], dt.bfloat16, tag="h1T")
        h2T = consts.tile([H, BL], dt.bfloat16, tag="h2T")
        hz = consts.tile([H, BL], dt.bfloat16, tag="hz")
        c0 = consts.tile([H, BL], dt.float32, tag="c0")
        c1 = consts.tile([H, BL], dt.float32, tag="c1")
        outs = consts.tile([16, BL], dt.float32, tag="outs")

        # input DMAs (xta split so the first GEMMs can start early)
        nxc = 8
        csz = (t_steps * BL) // nxc
        for i in range(nxc):
            nc.sync.dma_start(xta[:, i * csz:(i + 1) * csz], xta_d[:, i * csz:(i + 1) * csz])
        for tl, dr in ((w0aug, w0aug_d), (whh0, whh0_d), (w1, w1_d), (whh1, whh1_d),
                       (b1row, b1_d), (ind, ind_d), (wheadt, whead_d), (bhead, bhead_d)):
            nc.sync.dma_start(tl[:], dr[:])
        nc.vector.memset(hz[:], 0.0)

        psum0 = ctx.enter_context(tc.tile_pool(name="psum0", bufs=3, space=MS.PSUM))
        psum1 = ctx.enter_context(tc.tile_pool(name="psum1", bufs=3, space=MS.PSUM))
        psumh = ctx.enter_context(tc.tile_pool(name="psumh", bufs=1, space=MS.PSUM))
        sp = ctx.enter_context(tc.tile_pool(name="sp", bufs=4))
        tp = ctx.enter_context(tc.tile_pool(name="tp", bufs=4))

        banks = [None, None]  # live psum bank per layer

        # bank layout: col = j*128 + t_local*32 + b  (block-major so every
        # matmul output is a contiguous col range)
        def gemm_l0(k):
            bank = psum0.tile([H, 512], dt.float32, tag="bank0")
            banks[0] = bank
            rhs = xta[:, 4 * k * BL:(4 * k + 4) * BL]
            for j in range(4):
                nc.tensor.matmul(bank[:, j * H:(j + 1) * H], w0aug[:, j * H:(j + 1) * H],
                                 rhs, start=(j == 0), stop=False)

        def gemm_l1(k):
            bank = psum1.tile([H, 512], dt.float32, tag="bank1")
            banks[1] = bank
            nc.tensor.matmul(bank[:], b1row[:], ind[:], start=True, stop=False)
            rhs = h1T[:, 4 * k:4 * k + 4, :]
            for j in range(4):
                nc.tensor.matmul(bank[:, j * H:(j + 1) * H], w1[:, j * H:(j + 1) * H],
                                 rhs, start=False, stop=False)

        def step(layer, t):
            tl = t % 4
            bank = banks[layer]
            whh = whh0 if layer == 0 else whh1
            c = c0 if layer == 0 else c1
            if layer == 0:
                h_prev = hz if t == 0 else h1T[:, t - 1, :]
                h_out = h1T[:, t, :]
            else:
                h_prev = hz if t == 0 else h2T[:]
                h_out = h2T[:]
            base = tl * 32
            for j in range(4):
                nc.tensor.matmul(bank[:, j * H + base:j * H + base + 32],
                                 whh[:, j * H:(j + 1) * H], h_prev,
                                 start=False, stop=True)
            sig = sp.tile([H, 128], dt.float32, tag=f"sig{layer}")
            b4 = bank[:].rearrange("p (j x) -> p j x", j=4)
            nc.scalar.activation(sig[:, 0:96].rearrange("p (j x) -> p j x", j=3),
                                 b4[:, 0:3, base:base + 32], AF.Sigmoid)
            nc.scalar.activation(sig[:, 96:128], b4[:, 3, base:base + 32], AF.Tanh)
            if t == 0:
                # c(-1)=0: c = sig(i) * tanh(g)
                nc.vector.tensor_mul(c[:], sig[:, 0:32], sig[:, 96:128])
            else:
                tmp = tp.tile([H, BL], dt.float32, tag=f"tmp{layer}")
                nc.vector.tensor_mul(tmp[:], sig[:, 0:32], sig[:, 96:128])
                nc.vector.tensor_mul(c[:], sig[:, 32:64], c[:])
                nc.vector.tensor_add(c[:], c[:], tmp[:])
            th = tp.tile([H, BL], dt.float32, tag=f"th{layer}")
            nc.scalar.activation(th[:], c[:], AF.Tanh)
            nc.vector.tensor_mul(h_out, sig[:, 64:96], th[:])
            if dbg and layer == 0 and t == 0:
                nc.sync.dma_start(sig_d[:], sig[:])
            if dbg and layer == 0 and t == 3:
                sb = sp.tile([H, 512], dt.float32, tag="dbgbank")
                nc.vector.tensor_copy(sb[:], bank[:])
                nc.sync.dma_start(bank_d[:], sb[:])

        for k in range(NB + LAG):
            if k < NB:
                gemm_l0(k)
                for t in range(4 * k, 4 * k + 4):
                    step(0, t)
            if k >= LAG:
                kk = k - LAG
                gemm_l1(kk)
                for t in range(4 * kk, 4 * kk + 4):
                    step(1, t)

        if dbg:
            nc.sync.dma_start(h1_d[:], h1T[:])
        hp = psumh.tile([16, BL], dt.float32, tag="head")
        nc.tensor.matmul(hp[:], wheadt[:], h2T[:], start=True, stop=True)
        nc.scalar.activation(outs[:], hp[:], AF.Identity, bias=bhead[:, 0:1])
        nc.sync.dma_start(out_d[:], outs[:])

    nc.compile()
    return nc


def _pack_shared(W_ih0, W_hh0, b_ih0, b_hh0, W_ih1, W_hh1, b_ih1, b_hh1, W_head, b_head):
    bf16 = ml_dtypes.bfloat16
    b0 = (b_ih0 + b_hh0).astype(np.float32)
    b1 = (b_ih1 + b_hh1).astype(np.float32)

    w0aug = np.zeros((D + 1, 512), np.float32)
    whh0t = np.zeros((H, 512), np.float32)
    w1t = np.zeros((H, 512), np.float32)
    whh1t = np.zeros((H, 512), np.float32)
    b1row = np.zeros((4, H), np.float32)
    for j, g in enumerate(PERM):
        sl = slice(g * H, (g + 1) * H)
        w0aug[:D, j * H:(j + 1) * H] = W_ih0[sl].T
        w0aug[D, j * H:(j + 1) * H] = b0[sl]
        whh0t[:, j * H:(j + 1) * H] = W_hh0[sl].T
        w1t[:, j * H:(j + 1) * H] = W_ih1[sl].T
        whh1t[:, j * H:(j + 1) * H] = W_hh1[sl].T
        b1row[j] = b1[sl]

    ind = np.zeros((4, 512), np.float32)
    cols = np.arange(512)
    for r in range(4):
        ind[r] = (cols // 128 == r).astype(np.float32)

    wheadt = np.zeros((H, 16), np.float32)
    wheadt[:, :C] = W_head.T
    bhead = np.zeros((16, 1), np.float32)
    bhead[:C, 0] = b_head

    return {
        "w0aug": w0aug.astype(bf16), "whh0t": whh0t.astype(bf16),
        "w1t": w1t.astype(bf16), "whh1t": whh1t.astype(bf16),
        "b1row": b1row.astype(bf16), "ind": ind.astype(bf16),
        "wheadt": wheadt.astype(bf16), "bhead": bhead.astype(np.float32),
    }


def _make_xta(x_core, t_steps):
    # x_core [BL, T, D] -> [D+1, T*BL] with ones row (bias lane)
    bf16 = ml_dtypes.bfloat16
    xt = x_core[:, :t_steps, :].transpose(2, 1, 0).reshape(D, t_steps * BL)
    out = np.ones((D + 1, t_steps * BL), np.float32)
    out[:D] = xt
    return out.astype(bf16)


def run_cores(x, weights, t_steps=T, trace=False):
    from concourse.bass_utils import run_bass_kernel_spmd

    key = t_steps
    if key not in _cache:
        _cache[key] = _build_nc(t_steps)
    nc = _cache[key]

    shared = _pack_shared(**weights)
    in_maps = []
    for i in range(NCORES):
        m = dict(shared)
        m["xta"] = _make_xta(x[i * BL:(i + 1) * BL], t_steps)
        in_maps.append(m)
    res = run_bass_kernel_spmd(nc, in_maps, list(range(NCORES)), trace=trace)
    out = np.zeros((B, C), np.float32)
    for i in range(NCORES):
        out[i * BL:(i + 1) * BL] = res.results[i]["out"][:C, :].T
    return out, res


def kernel(x, W_ih0, W_hh0, b_ih0, b_hh0, W_ih1, W_hh1, b_ih1, b_hh1, W_head, b_head):
    weights = dict(W_ih0=W_ih0, W_hh0=W_hh0, b_ih0=b_ih0, b_hh0=b_hh0,
                   W_ih1=W_ih1, W_hh1=W_hh1, b_ih1=b_ih1, b_hh1=b_hh1,
                   W_head=W_head, b_head=b_head)
    weights = {k: np.asarray(v, np.float32) for k, v in weights.items()}
    out, _ = run_cores(np.asarray(x, np.float32), weights)
    return out



# revision 2
# speedup vs baseline: 1.2591x; 1.2591x over previous
"""Trainium2 Bass kernel for nn_LSTMClassifier (B=256,T=1024,D=64,H=128,C=10).

Data-parallel over batch across 8 cores (32 seqs/core), gate-major layout
(partitions = hidden units, batch on the free dim). The serial recurrence is
latency-bound, so the per-step chain is engineered to minimize serial
engine-visit latencies (2.48ms -> 2.05ms vs the 3-activation baseline).

Per-step critical path is latency-bound (serial recurrence), so the cell
update is restructured to minimize Activation-engine visits:
  - ONE Sigmoid instruction covers all 4 gates: the g-gate pre-activation
    is pre-scaled x2 in the packed weights, so sigma(2g) comes out and
    tanh(g) = 2*sigma(2g) - 1 is recovered on the DVE via fused
    scalar_tensor_tensor ops:
        t2 = (sig_g - 0.5) * sig_i          # = (i*tanh(g))/2
        t1 = sig_f * c
        c  = 2*t2 + t1
  - tanh(c) = 2*sigma(2c) - 1 via activation(scale=2.0); h is stored
    HALVED:  h_half = (sigma(2c) - 0.5) * sig_o  =  h/2
    with every consumer weight pre-doubled (W_hh same layer, W_ih1 for the
    next layer, W_head for the classifier head).
This gives 2 Act visits + 4 DVE ops per step instead of 3 Act + 4 DVE.
"""

import os
import sys

import numpy as np

for _p in ("/opt/trn_rl_repo",):
    if _p not in sys.path:
        sys.path.insert(0, _p)

import ml_dtypes  # noqa: E402

B, T, D, H, C = 256, 1024, 64, 128, 10
NCORES, BL = 8, 32
# column-block order [i, f, o, g]; reference split order is (i, f, g, o)
PERM = [0, 1, 3, 2]
GCOL = 3  # g-gate column block index in packed layout
LAG = 2  # banks (of 4 steps) that L1 trails L0 in program order

_cache = {}


def _build_nc(t_steps):
    from contextlib import ExitStack

    import concourse.bass as bass
    import concourse.mybir as mybir
    from concourse import bacc
    from concourse.tile import TileContext

    dt = mybir.dt
    AF = mybir.ActivationFunctionType
    ALU = mybir.AluOpType
    MS = bass.MemorySpace

    nc = bacc.Bacc(None, target_bir_lowering=False, debug=False)
    NB = t_steps // 4

    xta_d = nc.dram_tensor("xta", [D + 1, t_steps * BL], dt.bfloat16, kind="ExternalInput")
    w0aug_d = nc.dram_tensor("w0aug", [D + 1, 512], dt.bfloat16, kind="ExternalInput")
    whh0_d = nc.dram_tensor("whh0t", [H, 512], dt.bfloat16, kind="ExternalInput")
    w1_d = nc.dram_tensor("w1t", [H, 512], dt.bfloat16, kind="ExternalInput")
    whh1_d = nc.dram_tensor("whh1t", [H, 512], dt.bfloat16, kind="ExternalInput")
    b1_d = nc.dram_tensor("b1row", [4, H], dt.bfloat16, kind="ExternalInput")
    ind_d = nc.dram_tensor("ind", [4, 512], dt.bfloat16, kind="ExternalInput")
    whead_d = nc.dram_tensor("wheadt", [H, 16], dt.bfloat16, kind="ExternalInput")
    bhead_d = nc.dram_tensor("bhead", [16, 1], dt.float32, kind="ExternalInput")
    out_d = nc.dram_tensor("out", [16, BL], dt.float32, kind="ExternalOutput")

    with TileContext(nc) as tc, ExitStack() as ctx:
        consts = ctx.enter_context(tc.tile_pool(name="consts", bufs=1))
        xta = consts.tile([D + 1, t_steps * BL], dt.bfloat16, tag="xta")
        w0aug = consts.tile([D + 1, 512], dt.bfloat16, tag="w0aug")
        whh0 = consts.tile([H, 512], dt.bfloat16, tag="whh0")
        w1 = consts.tile([H, 512], dt.bfloat16, tag="w1")
        whh1 = consts.tile([H, 512], dt.bfloat16, tag="whh1")
        b1row = consts.tile([4, H], dt.bfloat16, tag="b1row")
        ind = consts.tile([4, 512], dt.bfloat16, tag="ind")
        wheadt = consts.tile([H, 16], dt.bfloat16, tag="wheadt")
        bhead = consts.tile([16, 1], dt.float32, tag="bhead")
        h1T = consts.tile([H, t_steps, BL], dt.bfloat16, tag="h1T")
        h2T = consts.tile([H, BL], dt.bfloat16, tag="h2T")
        hz = consts.tile([H, BL], dt.bfloat16, tag="hz")
        c0 = consts.tile([H, BL], dt.float32, tag="c0")
        c1 = consts.tile([H, BL], dt.float32, tag="c1")
        outs = consts.tile([16, BL], dt.float32, tag="outs")

        # input DMAs (xta split so the first GEMMs can start early)
        nxc = 16
        csz = (t_steps * BL) // nxc
        for i in range(nxc):
            nc.sync.dma_start(xta[:, i * csz:(i + 1) * csz], xta_d[:, i * csz:(i + 1) * csz])
        for tl, dr in ((w0aug, w0aug_d), (whh0, whh0_d), (w1, w1_d), (whh1, whh1_d),
                       (b1row, b1_d), (ind, ind_d), (wheadt, whead_d), (bhead, bhead_d)):
            nc.sync.dma_start(tl[:], dr[:])
        nc.vector.memset(hz[:], 0.0)

        psum0 = ctx.enter_context(tc.tile_pool(name="psum0", bufs=3, space=MS.PSUM))
        psum1 = ctx.enter_context(tc.tile_pool(name="psum1", bufs=3, space=MS.PSUM))
        psumh = ctx.enter_context(tc.tile_pool(name="psumh", bufs=1, space=MS.PSUM))
        sp = ctx.enter_context(tc.tile_pool(name="sp", bufs=4))
        tp = ctx.enter_context(tc.tile_pool(name="tp", bufs=6))

        banks = [None, None]  # live psum bank per layer

        # bank layout: col = j*128 + t_local*32 + b  (block-major so every
        # matmul output is a contiguous col range)
        def gemm_l0(k):
            bank = psum0.tile([H, 512], dt.float32, tag="bank0")
            banks[0] = bank
            rhs = xta[:, 4 * k * BL:(4 * k + 4) * BL]
            for j in range(4):
                nc.tensor.matmul(bank[:, j * H:(j + 1) * H], w0aug[:, j * H:(j + 1) * H],
                                 rhs, start=(j == 0), stop=False)

        def gemm_l1(k):
            bank = psum1.tile([H, 512], dt.float32, tag="bank1")
            banks[1] = bank
            nc.tensor.matmul(bank[:], b1row[:], ind[:], start=True, stop=False)
            rhs = h1T[:, 4 * k:4 * k + 4, :]
            for j in range(4):
                nc.tensor.matmul(bank[:, j * H:(j + 1) * H], w1[:, j * H:(j + 1) * H],
                                 rhs, start=False, stop=False)

        def step(layer, t):
            tl = t % 4
            bank = banks[layer]
            whh = whh0 if layer == 0 else whh1
            c = c0 if layer == 0 else c1
            if layer == 0:
                h_prev = hz if t == 0 else h1T[:, t - 1, :]
                h_out = h1T[:, t, :]
            else:
                h_prev = hz if t == 0 else h2T[:]
                h_out = h2T[:]
            base = tl * 32
            # recurrent gate matmuls (whh pre-doubled: h stored halved)
            for j in range(4):
                nc.tensor.matmul(bank[:, j * H + base:j * H + base + 32],
                                 whh[:, j * H:(j + 1) * H], h_prev,
                                 start=False, stop=True)
            # ONE sigmoid over all 4 gate blocks (g pre-scaled x2 so
            # sig[:,3] = sigma(2*zg) and tanh(zg) = 2*sig[:,3]-1)
            sig = sp.tile([H, 128], dt.float32, tag=f"sig{layer}")
            b4 = bank[:].rearrange("p (j x) -> p j x", j=4)
            nc.scalar.activation(sig[:].rearrange("p (j x) -> p j x", j=4),
                                 b4[:, :, base:base + 32], AF.Sigmoid)
            si, sf, so, sg = (sig[:, 0:32], sig[:, 32:64], sig[:, 64:96], sig[:, 96:128])
            # c update: c = 2*((sg-0.5)*si) + sf*c   [= f*c + i*tanh(zg)]
            # t1 issued FIRST: t1 and t2 are independent, so they pipeline in
            # the DVE; c waits only the later drain.
            t2 = tp.tile([H, BL], dt.float32, tag=f"t2{layer}")
            if t == 0:
                nc.vector.scalar_tensor_tensor(t2[:], sg, 0.5, si, ALU.subtract, ALU.mult)
                nc.vector.tensor_scalar_mul(c[:], t2[:], 2.0)
            else:
                t1 = tp.tile([H, BL], dt.float32, tag=f"t1{layer}")
                nc.vector.tensor_mul(t1[:], sf, c[:])
                nc.vector.scalar_tensor_tensor(t2[:], sg, 0.5, si, ALU.subtract, ALU.mult)
                nc.vector.scalar_tensor_tensor(c[:], t2[:], 2.0, t1[:], ALU.mult, ALU.add)
            # h/2 = (sigma(2c)-0.5)*o ; consumers' weights are pre-doubled
            sc = tp.tile([H, BL], dt.float32, tag=f"sc{layer}")
            nc.scalar.activation(sc[:], c[:], AF.Sigmoid, scale=2.0)
            nc.vector.scalar_tensor_tensor(h_out, sc[:], 0.5, so, ALU.subtract, ALU.mult)

        for k in range(NB + LAG):
            if k < NB:
                gemm_l0(k)
                for t in range(4 * k, 4 * k + 4):
                    step(0, t)
            if k >= LAG:
                kk = k - LAG
                gemm_l1(kk)
                for t in range(4 * kk, 4 * kk + 4):
                    step(1, t)

        hp = psumh.tile([16, BL], dt.float32, tag="head")
        nc.tensor.matmul(hp[:], wheadt[:], h2T[:], start=True, stop=True)
        nc.scalar.activation(outs[:], hp[:], AF.Identity, bias=bhead[:, 0:1])
        nc.sync.dma_start(out_d[:], outs[:])

    nc.compile()
    return nc


def _pack_shared(W_ih0, W_hh0, b_ih0, b_hh0, W_ih1, W_hh1, b_ih1, b_hh1, W_head, b_head):
    bf16 = ml_dtypes.bfloat16
    b0 = (b_ih0 + b_hh0).astype(np.float32)
    b1 = (b_ih1 + b_hh1).astype(np.float32)

    w0aug = np.zeros((D + 1, 512), np.float32)
    whh0t = np.zeros((H, 512), np.float32)
    w1t = np.zeros((H, 512), np.float32)
    whh1t = np.zeros((H, 512), np.float32)
    b1row = np.zeros((4, H), np.float32)
    for j, g in enumerate(PERM):
        sl = slice(g * H, (g + 1) * H)
        # gate scale: g-gate pre-activation doubled (sigmoid trick)
        gs = 2.0 if j == GCOL else 1.0
        # W_ih0 consumes x directly (unhalved input)
        w0aug[:D, j * H:(j + 1) * H] = W_ih0[sl].T * gs
        w0aug[D, j * H:(j + 1) * H] = b0[sl] * gs
        # recurrent + inter-layer weights consume h/2 -> doubled, then
        # g-gate doubled again
        whh0t[:, j * H:(j + 1) * H] = W_hh0[sl].T * (2.0 * gs)
        w1t[:, j * H:(j + 1) * H] = W_ih1[sl].T * (2.0 * gs)
        whh1t[:, j * H:(j + 1) * H] = W_hh1[sl].T * (2.0 * gs)
        b1row[j] = b1[sl] * gs

    ind = np.zeros((4, 512), np.float32)
    cols = np.arange(512)
    for r in range(4):
        ind[r] = (cols // 128 == r).astype(np.float32)

    wheadt = np.zeros((H, 16), np.float32)
    wheadt[:, :C] = W_head.T * 2.0  # consumes h2/2
    bhead = np.zeros((16, 1), np.float32)
    bhead[:C, 0] = b_head

    return {
        "w0aug": w0aug.astype(bf16), "whh0t": whh0t.astype(bf16),
        "w1t": w1t.astype(bf16), "whh1t": whh1t.astype(bf16),
        "b1row": b1row.astype(bf16), "ind": ind.astype(bf16),
        "wheadt": wheadt.astype(bf16), "bhead": bhead.astype(np.float32),
    }


def _make_xta(x_core, t_steps):
    # x_core [BL, T, D] -> [D+1, T*BL] with ones row (bias lane)
    bf16 = ml_dtypes.bfloat16
    xt = x_core[:, :t_steps, :].transpose(2, 1, 0).reshape(D, t_steps * BL)
    out = np.ones((D + 1, t_steps * BL), np.float32)
    out[:D] = xt
    return out.astype(bf16)


def run_cores(x, weights, t_steps=T, trace=False):
    from concourse.bass_utils import run_bass_kernel_spmd

    key = t_steps
    if key not in _cache:
        _cache[key] = _build_nc(t_steps)
    nc = _cache[key]

    shared = _pack_shared(**weights)
    in_maps = []
    for i in range(NCORES):
        m = dict(shared)
        m["xta"] = _make_xta(x[i * BL:(i + 1) * BL], t_steps)
        in_maps.append(m)
    res = run_bass_kernel_spmd(nc, in_maps, list(range(NCORES)), trace=trace)
    out = np.zeros((B, C), np.float32)
    for i in range(NCORES):
        out[i * BL:(i + 1) * BL] = res.results[i]["out"][:C, :].T
    return out, res


def kernel(x, W_ih0, W_hh0, b_ih0, b_hh0, W_ih1, W_hh1, b_ih1, b_hh1, W_head, b_head):
    weights = dict(W_ih0=W_ih0, W_hh0=W_hh0, b_ih0=b_ih0, b_hh0=b_hh0,
                   W_ih1=W_ih1, W_hh1=W_hh1, b_ih1=b_ih1, b_hh1=b_hh1,
                   W_head=W_head, b_head=b_head)
    weights = {k: np.asarray(v, np.float32) for k, v in weights.items()}
    out, _ = run_cores(np.asarray(x, np.float32), weights)
    return out
